# revision 1
# baseline (speedup 1.0000x reference)
"""Trainium2 Bass kernel for nn_MetaNet (triu-gram features -> Wh matvec ->
relu -> 14 per-head linears), distributed over 8 NeuronCores.

Sharding (uniform SPMD program, all per-core differences live in input data):
  stage 1: Wh sharded by hidden rows (224/core). Every core assembles the
           full 76128-dim feat vector on device (avgpool g4 + per-row triu
           DMA copies into a DRAM scratch, then one reshape DMA into SBUF
           [128, 595]); streams its own [128, 224] Wh tiles as the moving
           matmul operand with the feat chunk stationary; partial hidden
           [1, 224] accumulates in PSUM over 595 matmuls.
  sync:    AllGather(224 floats/core -> 1792), + bh, relu -> hstat [128, 14].
  stage 2: flattened Wf (14*36864 rows) split into 1008 [128, 512] tiles,
           126/core. Each tile computes out[14, 512] = hstat.T @ wf_tile
           for ALL 14 heads (same PE cost as one head: cost ~ N columns);
           the host selects the correct head row while unsharding.
"""

import math
from dataclasses import dataclass, field

import numpy as np


@dataclass(frozen=True)
class Cfg:
    n_cores: int = 8
    # triu source sizes: g1, g2, g3, pooled g4 (g4 input is 2x this)
    cs: tuple = (64, 128, 256, 256)
    hid: int = 128          # per-head hidden width (partition dim of hstat)
    nl: int = 14            # number of heads
    d2: int = 36864         # per-head fc output width
    tile_n2: int = 512      # stage-2 moving-operand width (<=512 fp32)

    @property
    def feat_len(self):
        return sum(c * (c + 1) // 2 for c in self.cs)

    @property
    def nk(self):  # number of 128-row feat chunks
        return math.ceil(self.feat_len / 128)

    @property
    def hidden(self):
        return self.hid * self.nl

    @property
    def rows(self):  # hidden rows per core
        assert self.hidden % self.n_cores == 0
        return self.hidden // self.n_cores

    @property
    def t2(self):  # stage-2 tiles per core
        total = self.nl * self.d2
        assert total % (self.n_cores * self.tile_n2) == 0
        assert self.d2 % self.tile_n2 == 0
        return total // (self.n_cores * self.tile_n2)

    @property
    def tri_offs(self):
        offs, base = [], 0
        for c in self.cs:
            offs.append(base)
            base += c * (c + 1) // 2
        return offs


FULL = Cfg()


def build_nc(cfg: Cfg, debug_taps: bool = False):
    import concourse.bacc as bacc
    import concourse.tile as tile
    import concourse.mybir as mybir

    f32 = mybir.dt.float32
    nc = bacc.Bacc("TRN2", target_bir_lowering=False, debug=False,
                   num_devices=cfg.n_cores)

    c1, c2, c3, c4 = cfg.cs
    g1 = nc.dram_tensor("g1", [c1, c1], f32, kind="ExternalInput")
    g2 = nc.dram_tensor("g2", [c2, c2], f32, kind="ExternalInput")
    g3 = nc.dram_tensor("g3", [c3, c3], f32, kind="ExternalInput")
    g4 = nc.dram_tensor("g4", [2 * c4, 2 * c4], f32, kind="ExternalInput")
    wh = nc.dram_tensor("wh", [cfg.nk, 128, cfg.rows], f32,
                        kind="ExternalInput")
    wf = nc.dram_tensor("wf", [cfg.t2, cfg.hid, cfg.tile_n2], f32,
                        kind="ExternalInput")
    bfv = nc.dram_tensor("bfv", [cfg.t2, cfg.tile_n2], f32,
                         kind="ExternalInput")
    bh = nc.dram_tensor("bh", [cfg.hidden], f32, kind="ExternalInput")
    out = nc.dram_tensor("out", [cfg.t2, cfg.nl, cfg.tile_n2], f32,
                         kind="ExternalOutput")
    if debug_taps:
        dbg_feat = nc.dram_tensor("dbg_feat", [128, cfg.nk], f32,
                                  kind="ExternalOutput")
        dbg_hpart = nc.dram_tensor("dbg_hpart", [cfg.rows], f32,
                                   kind="ExternalOutput")
        dbg_hstat = nc.dram_tensor("dbg_hstat", [cfg.hid, cfg.nl], f32,
                                   kind="ExternalOutput")

    with tile.TileContext(nc) as tc:
        with (
            tc.tile_pool(name="const", bufs=1) as const,
            tc.tile_pool(name="poolq", bufs=1) as poolq,
            tc.tile_pool(name="whp", bufs=24) as whp,
            tc.tile_pool(name="wfp", bufs=12) as wfp,
            tc.tile_pool(name="bfp", bufs=12) as bfp,
            tc.tile_pool(name="outp", bufs=12) as outp,
            tc.tile_pool(name="ps1", bufs=1, space="PSUM") as ps1p,
            tc.tile_pool(name="ps2", bufs=6, space="PSUM") as ps2p,
            tc.tile_pool(name="dram", bufs=1, space="DRAM") as dram,
        ):
            # ---- avgpool g4 [2C,2C] -> pooled [C,C] held in SBUF chunks ----
            # quadrant views: g4[a::2, b::2] for a,b in {0,1}
            g4q = g4[:].rearrange("(r a) (c b) -> a b r c", a=2, b=2)
            n_pch = math.ceil(c4 / 128)
            pool_tiles = []
            for h in range(n_pch):
                r0, r1 = h * 128, min(c4, (h + 1) * 128)
                pr = r1 - r0
                quads = []
                for a in range(2):
                    for b in range(2):
                        q = poolq.tile([pr, c4], f32, tag=f"q{a}{b}")
                        nc.sync.dma_start(q[:], g4q[a, b, r0:r1, :])
                        quads.append(q)
                s01 = poolq.tile([pr, c4], f32, tag="s01")
                s23 = poolq.tile([pr, c4], f32, tag="s23")
                nc.vector.tensor_add(s01[:], quads[0][:], quads[1][:])
                nc.vector.tensor_add(s23[:], quads[2][:], quads[3][:])
                s = poolq.tile([pr, c4], f32, tag="s")
                nc.vector.tensor_add(s[:], s01[:], s23[:])
                pt = const.tile([pr, c4], f32, tag=f"pool{h}")
                nc.scalar.mul(pt[:], s[:], 0.25)
                pool_tiles.append(pt)

            # ---- assemble packed triu feat grid directly in SBUF ----
            # cell g (= flat feat index) lives at featk[g // nk, g % nk];
            # host permutes Wh columns to match (wh tile k row r <-> cell
            # r*nk + k). Triu rows are contiguous flat runs -> each row is
            # 1-4 small SBUF writes, all dependency-tracked by Tile.
            nk = cfg.nk
            featk = const.tile([128, nk], f32)
            nc.gpsimd.memset(featk[:], 0.0)
            offs = cfg.tri_offs
            for m, (c, src) in enumerate(zip(cfg.cs, [g1, g2, g3, None])):
                base = offs[m]
                for y in range(c):
                    off = base + y * c - y * (y - 1) // 2
                    ln = c - y
                    done = 0
                    while done < ln:
                        g = off + done
                        p, j = g // nk, g % nk
                        seg = min(ln - done, nk - j)
                        if src is not None:
                            nc.sync.dma_start(
                                featk[p:p + 1, j:j + seg],
                                src[y, y + done:y + done + seg])
                        else:
                            pt = pool_tiles[y // 128]
                            yp = y % 128
                            nc.sync.dma_start(
                                featk[p:p + 1, j:j + seg],
                                pt[yp:yp + 1, y + done:y + done + seg])
                        done += seg

            # bias for hidden, laid out to match hstat [hid, nl]
            bh_t = const.tile([cfg.hid, cfg.nl], f32)
            nc.sync.dma_start(
                bh_t[:], bh[:].rearrange("(n p) -> p n", p=cfg.hid))

            # ---- stage 1: partial hidden [1, rows] over 595 feat chunks ----
            psum1 = ps1p.tile([1, cfg.rows], f32)
            for k in range(cfg.nk):
                whk = whp.tile([128, cfg.rows], f32, tag="whk")
                nc.sync.dma_start(whk[:], wh[k])
                nc.tensor.matmul(psum1[:], featk[:, k:k + 1], whk[:],
                                 start=(k == 0), stop=(k == cfg.nk - 1))

            hpart = const.tile([1, cfg.rows], f32)
            nc.vector.tensor_copy(hpart[:], psum1[:])
            cc_in = dram.tile([1, cfg.rows], f32)
            nc.sync.dma_start(cc_in[:], hpart[:])
            cc_out = dram.tile([cfg.n_cores, cfg.rows], f32)
            nc.gpsimd.collective_compute(
                "AllGather", mybir.AluOpType.bypass,
                replica_groups=[list(range(cfg.n_cores))],
                ins=[cc_in[:].opt()], outs=[cc_out[:].opt()],
            )
            hraw = const.tile([cfg.hid, cfg.nl], f32)
            nc.sync.dma_start(
                hraw[:],
                cc_out[:].rearrange("a b -> (a b)").rearrange(
                    "(n p) -> p n", p=cfg.hid))
            hsum = const.tile([cfg.hid, cfg.nl], f32)
            nc.vector.tensor_add(hsum[:], hraw[:], bh_t[:])
            hstat = const.tile([cfg.hid, cfg.nl], f32)
            nc.scalar.activation(hstat[:], hsum[:],
                                 mybir.ActivationFunctionType.Relu)

            ones1 = const.tile([1, cfg.nl], f32)
            nc.gpsimd.memset(ones1[:], 1.0)

            if debug_taps:
                nc.sync.dma_start(dbg_feat[:], featk[:])
                nc.sync.dma_start(dbg_hpart[:], hpart[0, :])
                nc.sync.dma_start(dbg_hstat[:], hstat[:])

            # ---- stage 2: per tile, out[nl, tile_n2] for all heads ----
            for u in range(cfg.t2):
                wfu = wfp.tile([cfg.hid, cfg.tile_n2], f32, tag="wfu")
                nc.sync.dma_start(wfu[:], wf[u])
                bfu = bfp.tile([1, cfg.tile_n2], f32, tag="bfu")
                nc.sync.dma_start(bfu[:], bfv[u])
                ps2 = ps2p.tile([cfg.nl, cfg.tile_n2], f32, tag="ps2")
                nc.tensor.matmul(ps2[:], hstat[:], wfu[:],
                                 start=True, stop=False)
                nc.tensor.matmul(ps2[:], ones1[:], bfu[:],
                                 start=False, stop=True)
                ot = outp.tile([cfg.nl, cfg.tile_n2], f32, tag="ot")
                if u % 2 == 0:
                    nc.vector.tensor_copy(ot[:], ps2[:])
                else:
                    nc.scalar.copy(ot[:], ps2[:])
                nc.sync.dma_start(out[u], ot[:])

    nc.compile()
    return nc


def shard_inputs(cfg: Cfg, g1, g2, g3, g4, Wh, bh, Wf, bf):
    """Full inputs -> list of per-core in_maps (numpy, contiguous)."""
    f32 = np.float32
    g1 = np.ascontiguousarray(g1.reshape(cfg.cs[0], cfg.cs[0]), dtype=f32)
    g2 = np.ascontiguousarray(g2.reshape(cfg.cs[1], cfg.cs[1]), dtype=f32)
    g3 = np.ascontiguousarray(g3.reshape(cfg.cs[2], cfg.cs[2]), dtype=f32)
    g4 = np.ascontiguousarray(
        g4.reshape(2 * cfg.cs[3], 2 * cfg.cs[3]), dtype=f32)
    bh = np.ascontiguousarray(bh.reshape(cfg.hidden), dtype=f32)

    ntile_per_head = cfg.d2 // cfg.tile_n2
    # global stage-2 tiles: [nl*d2/tile_n2, hid, tile_n2]
    Wfr = Wf.reshape(cfg.nl * ntile_per_head, cfg.tile_n2, cfg.hid)
    bfr = bf.reshape(cfg.nl * ntile_per_head, cfg.tile_n2)

    in_maps = []
    for c in range(cfg.n_cores):
        whc = Wh[c * cfg.rows:(c + 1) * cfg.rows, :]  # [rows, feat_len]
        whp = np.zeros((cfg.rows, cfg.nk * 128), dtype=f32)
        whp[:, :cfg.feat_len] = whc
        # feat cell g = p*nk + j maps to featk[p, j]; wh tile k row r
        # multiplies featk[r, k] = feat cell r*nk + k
        wh_tiles = np.ascontiguousarray(
            whp.reshape(cfg.rows, 128, cfg.nk).transpose(2, 1, 0))

        t0 = c * cfg.t2
        wf_tiles = np.ascontiguousarray(
            Wfr[t0:t0 + cfg.t2].transpose(0, 2, 1), dtype=f32)
        bf_tiles = np.ascontiguousarray(bfr[t0:t0 + cfg.t2], dtype=f32)

        in_maps.append({
            "g1": g1, "g2": g2, "g3": g3, "g4": g4,
            "wh": wh_tiles, "wf": wf_tiles, "bfv": bf_tiles, "bh": bh,
        })
    return in_maps


def unshard_output(cfg: Cfg, outs):
    """outs: list of per-core [t2, nl, tile_n2] -> [nl, 1, d2]."""
    glob = np.concatenate(outs, axis=0)  # [total_tiles, nl, tile_n2]
    total = glob.shape[0]
    ntile_per_head = cfg.d2 // cfg.tile_n2
    n_idx = np.arange(total) // ntile_per_head
    sel = glob[np.arange(total), n_idx, :]  # [total_tiles, tile_n2]
    return np.ascontiguousarray(
        sel.reshape(cfg.nl, cfg.d2)[:, None, :])


_NC_CACHE = {}


def _get_nc(cfg: Cfg):
    if cfg not in _NC_CACHE:
        _NC_CACHE[cfg] = build_nc(cfg)
    return _NC_CACHE[cfg]


def kernel(g1, g2, g3, g4, Wh, bh, Wf, bf):
    from concourse import bass_utils

    cfg = FULL
    nc = _get_nc(cfg)
    in_maps = shard_inputs(cfg, g1, g2, g3, g4, Wh, bh, Wf, bf)
    res = bass_utils.run_bass_kernel_spmd(
        nc, in_maps, core_ids=list(range(cfg.n_cores)))
    return unshard_output(cfg, [res.results[c]["out"]
                                for c in range(cfg.n_cores)])



# revision 2
# speedup vs baseline: 1.0086x; 1.0086x over previous
"""Trainium2 Bass kernel for nn_MetaNet (triu-gram features -> Wh matvec ->
relu -> 14 per-head linears), distributed over 8 NeuronCores.

v5 = v3 + Shared-DRAM collective output.

v3 — fixes the two stage-2 serial bottlenecks found in the v2 trace
(126 single-lane DVE psum drains @795ns + 126 small out-DMAs each costing
~700ns of Sync-engine issue time) and the DMA stall during the AllGather:

  * Stage 1 (unchanged math): feat grid [128,595] bf16 in SBUF, Wh streams
    as 17 DMAs of [128, 35*224] bf16 (~1.96MB each), 595 accumulating
    matmuls (moving operand = Wh slice, N=224) into psum [1,224].
  * Collective-dependent DMAs (cc_in store, hidden reload) are issued from
    GpSimd (SWDGE) so the Sync-engine HWDGE FIFO (wh then wf streams)
    never blocks on the gather -> Wf prefetch fills the ~35us gather
    bubble (wfp bufs=21 holds the whole per-core Wf shard, 16.5MB).
  * Stage 2 transposed matmuls: out_chunk[128,1] = wf[128,128chunk].T @
    hsel[:,j] with N=1, M=128. All 504 output columns land in ONE psum
    bank [128,504]; a single DVE copy + a single 258KB DMA drain the
    whole stage-2 output (engages all 128 DVE lanes instead of 1).
"""

import math
from dataclasses import dataclass

import numpy as np
import ml_dtypes

BF16 = ml_dtypes.bfloat16


@dataclass(frozen=True)
class Cfg:
    n_cores: int = 8
    cs: tuple = (64, 128, 256, 256)
    hid: int = 128
    nl: int = 14
    d2: int = 36864
    tile_n2: int = 512      # output cols per stage-2 tile
    qn: int = 128           # output cols per stage-2 matmul (psum column)
    g1k: int = 35           # stage-1 feat chunks per DMA group
    g2k: int = 6            # stage-2 tiles per DMA group
    run: int = 18           # stage-2 tiles per head-run (126 = 7*18)

    @property
    def feat_len(self):
        return sum(c * (c + 1) // 2 for c in self.cs)  # 76128

    @property
    def nk(self):
        return math.ceil(self.feat_len / 128)  # 595

    @property
    def hidden(self):
        return self.hid * self.nl  # 1792

    @property
    def rows(self):
        assert self.hidden % self.n_cores == 0
        return self.hidden // self.n_cores  # 224

    @property
    def ng1(self):
        assert self.nk % self.g1k == 0
        return self.nk // self.g1k  # 17

    @property
    def t2(self):
        total = self.nl * self.d2
        assert total % (self.n_cores * self.tile_n2) == 0
        return total // (self.n_cores * self.tile_n2)  # 126

    @property
    def ng2(self):
        assert self.t2 % self.g2k == 0
        return self.t2 // self.g2k  # 21

    @property
    def nruns(self):
        assert self.t2 % self.run == 0
        return self.t2 // self.run  # 7

    @property
    def nq(self):  # stage-2 matmuls per tile
        assert self.tile_n2 % self.qn == 0
        return self.tile_n2 // self.qn  # 4

    @property
    def outc(self):  # total stage-2 psum columns (<= 512, one bank)
        v = self.t2 * self.nq  # 504
        assert v <= 512
        return v

    @property
    def ntile_per_head(self):
        assert self.d2 % self.tile_n2 == 0
        return self.d2 // self.tile_n2  # 72


FULL = Cfg()


def build_nc(cfg: Cfg, debug_taps: bool = False):
    import concourse.bacc as bacc
    import concourse.tile as tile
    import concourse.mybir as mybir

    f32 = mybir.dt.float32
    bf16 = mybir.dt.bfloat16
    nc = bacc.Bacc("TRN2", target_bir_lowering=False, debug=False,
                   num_devices=cfg.n_cores)

    featg = nc.dram_tensor("featg", [128, cfg.nk], bf16, kind="ExternalInput")
    wh = nc.dram_tensor("wh", [cfg.ng1, 128, cfg.g1k * cfg.rows], bf16,
                        kind="ExternalInput")
    bht = nc.dram_tensor("bht", [cfg.nl, cfg.hid], f32, kind="ExternalInput")
    sel = nc.dram_tensor("sel", [cfg.nl, cfg.nruns], bf16,
                         kind="ExternalInput")
    wf = nc.dram_tensor("wf", [cfg.ng2, cfg.hid, cfg.g2k * cfg.tile_n2],
                        bf16, kind="ExternalInput")
    out = nc.dram_tensor("out", [128, cfg.outc], f32, kind="ExternalOutput")
    if debug_taps:
        dbg_hpart = nc.dram_tensor("dbg_hpart", [cfg.rows], f32,
                                   kind="ExternalOutput")
        dbg_hsel = nc.dram_tensor("dbg_hsel", [cfg.hid, cfg.nruns], f32,
                                  kind="ExternalOutput")

    with tile.TileContext(nc) as tc:
        with (
            tc.tile_pool(name="const", bufs=1) as const,
            tc.tile_pool(name="whp", bufs=3) as whp,
            tc.tile_pool(name="wfp", bufs=21) as wfp,
            tc.tile_pool(name="ps1", bufs=1, space="PSUM") as ps1p,
            tc.tile_pool(name="psh", bufs=1, space="PSUM") as pshp,
            tc.tile_pool(name="ps2", bufs=1, space="PSUM") as ps2p,
            tc.tile_pool(name="dram", bufs=1, space="DRAM") as dram,
        ):
            featg_t = const.tile([128, cfg.nk], bf16)
            nc.sync.dma_start(featg_t[:], featg[:])
            bht_t = const.tile([cfg.nl, cfg.hid], f32)
            nc.sync.dma_start(bht_t[:], bht[:])
            sel_t = const.tile([cfg.nl, cfg.nruns], bf16)
            nc.sync.dma_start(sel_t[:], sel[:])

            # ---- stage 1: partial hidden [1, rows] over 595 feat chunks ----
            psum1 = ps1p.tile([1, cfg.rows], f32)
            for g in range(cfg.ng1):
                whg = whp.tile([128, cfg.g1k * cfg.rows], bf16, tag="whg")
                nc.sync.dma_start(whg[:], wh[g])
                for t in range(cfg.g1k):
                    k = g * cfg.g1k + t
                    nc.tensor.matmul(
                        psum1[:], featg_t[:, k:k + 1],
                        whg[:, t * cfg.rows:(t + 1) * cfg.rows],
                        start=(k == 0), stop=(k == cfg.nk - 1))

            hpart = const.tile([1, cfg.rows], f32)
            nc.vector.tensor_copy(hpart[:], psum1[:])
            # gather-dependent DMAs go via GpSimd/SWDGE so the Sync HWDGE
            # FIFO (carrying the wf prefetch stream) never stalls on them.
            cc_in = dram.tile([1, cfg.rows], f32)
            nc.gpsimd.dma_start(cc_in[:], hpart[:])
            cc_out = dram.tile([cfg.n_cores, cfg.rows], f32,
                               addr_space="Shared")
            nc.gpsimd.collective_compute(
                "AllGather", mybir.AluOpType.bypass,
                replica_groups=[list(range(cfg.n_cores))],
                ins=[cc_in[:].opt()], outs=[cc_out[:].opt()],
            )
            hraw = const.tile([cfg.nl, cfg.hid], f32)
            nc.gpsimd.dma_start(
                hraw[:],
                cc_out[:].rearrange("a b -> (a b)").rearrange(
                    "(n p) -> n p", p=cfg.hid))
            hsum = const.tile([cfg.nl, cfg.hid], f32)
            nc.vector.tensor_add(hsum[:], hraw[:], bht_t[:])
            hstat = const.tile([cfg.nl, cfg.hid], bf16)
            nc.scalar.activation(hstat[:], hsum[:],
                                 mybir.ActivationFunctionType.Relu)

            psumh = pshp.tile([cfg.hid, cfg.nruns], f32)
            nc.tensor.matmul(psumh[:], hstat[:], sel_t[:],
                             start=True, stop=True)
            hsel = const.tile([cfg.hid, cfg.nruns], bf16)
            nc.vector.tensor_copy(hsel[:], psumh[:])

            if debug_taps:
                nc.sync.dma_start(dbg_hpart[:], hpart[0, :])
                nc.sync.dma_start(dbg_hsel[:], psumh[:])

            # ---- stage 2: 504 transposed matmuls into one psum bank ----
            ps2 = ps2p.tile([128, cfg.outc], f32)
            for j in range(cfg.nruns):
                for t in range(cfg.run):
                    u = j * cfg.run + t
                    g, s = divmod(u, cfg.g2k)
                    if s == 0:
                        wfg = wfp.tile([cfg.hid, cfg.g2k * cfg.tile_n2],
                                       bf16, tag="wfg")
                        nc.sync.dma_start(wfg[:], wf[g])
                    for q in range(cfg.nq):
                        c = u * cfg.nq + q
                        off = s * cfg.tile_n2 + q * cfg.qn
                        nc.tensor.matmul(
                            ps2[:, c:c + 1],
                            wfg[:, off:off + cfg.qn],
                            hsel[:, j:j + 1],
                            start=True, stop=True)
            ot = const.tile([128, cfg.outc], f32)
            nc.vector.tensor_copy(ot[:], ps2[:])
            nc.sync.dma_start(out[:], ot[:])

    nc.compile()
    return nc


def _assemble_feat(cfg: Cfg, g1, g2, g3, g4):
    """avgpool(g4) + packed triu features -> bf16 grid [128, nk]."""
    g4 = g4.reshape(2 * cfg.cs[3], 2 * cfg.cs[3]).astype(np.float32)
    g4p = g4.reshape(cfg.cs[3], 2, cfg.cs[3], 2).mean(axis=(1, 3))
    parts = []
    for c, g in zip(cfg.cs, [g1, g2, g3, g4p]):
        m = np.asarray(g).reshape(c, c)
        r, co = np.triu_indices(c)
        parts.append(m[r, co])
    feat = np.concatenate(parts).astype(np.float32)
    assert feat.shape[0] == cfg.feat_len
    grid = np.zeros(128 * cfg.nk, dtype=np.float32)
    grid[:cfg.feat_len] = feat
    return np.ascontiguousarray(grid.reshape(128, cfg.nk)).astype(BF16)


def shard_inputs(cfg: Cfg, g1, g2, g3, g4, Wh, bh, Wf, bf):
    """Full inputs -> list of per-core in_maps (numpy, contiguous)."""
    featg = _assemble_feat(cfg, g1, g2, g3, g4)
    bht = np.ascontiguousarray(
        np.asarray(bh, dtype=np.float32).reshape(cfg.nl, cfg.hid))

    Wh = np.asarray(Wh, dtype=np.float32)
    Wfr = np.asarray(Wf, dtype=np.float32).reshape(
        cfg.nl * cfg.ntile_per_head, cfg.tile_n2, cfg.hid)

    in_maps = []
    for c in range(cfg.n_cores):
        whc = Wh[c * cfg.rows:(c + 1) * cfg.rows, :]
        whp = np.zeros((cfg.rows, cfg.nk * 128), dtype=np.float32)
        whp[:, :cfg.feat_len] = whc
        wh_tiles = whp.reshape(cfg.rows, 128, cfg.nk).transpose(2, 1, 0)
        wh_g = np.ascontiguousarray(
            wh_tiles.reshape(cfg.ng1, cfg.g1k, 128, cfg.rows)
            .transpose(0, 2, 1, 3)
            .reshape(cfg.ng1, 128, cfg.g1k * cfg.rows)).astype(BF16)

        t0 = c * cfg.t2
        wf_tiles = Wfr[t0:t0 + cfg.t2].transpose(0, 2, 1)  # [t2, hid, n2]
        wf_g = np.ascontiguousarray(
            wf_tiles.reshape(cfg.ng2, cfg.g2k, cfg.hid, cfg.tile_n2)
            .transpose(0, 2, 1, 3)
            .reshape(cfg.ng2, cfg.hid, cfg.g2k * cfg.tile_n2)).astype(BF16)

        selm = np.zeros((cfg.nl, cfg.nruns), dtype=np.float32)
        for j in range(cfg.nruns):
            h = (t0 + j * cfg.run) // cfg.ntile_per_head
            selm[h, j] = 1.0

        in_maps.append({
            "featg": featg, "wh": wh_g, "bht": bht,
            "sel": selm.astype(BF16), "wf": wf_g,
        })
    return in_maps


def unshard_output(cfg: Cfg, outs, bf):
    """outs: list of per-core [128, outc] -> [nl, 1, d2] (+ bf)."""
    # column c = u*nq + q holds output values [q*128 .. q*128+128) of
    # per-core tile u; row p is the value index within the chunk.
    parts = []
    for arr in outs:
        a = np.asarray(arr, dtype=np.float32).T  # [outc, 128]
        parts.append(a.reshape(cfg.t2, cfg.nq * 128))  # [126, 512]
    glob = np.concatenate(parts, axis=0)  # [1008, 512]
    res = glob.reshape(cfg.nl, cfg.d2)
    res = res + np.asarray(bf, dtype=np.float32).reshape(cfg.nl, cfg.d2)
    return np.ascontiguousarray(res[:, None, :])


_NC_CACHE = {}


def _get_nc(cfg: Cfg):
    if cfg not in _NC_CACHE:
        _NC_CACHE[cfg] = build_nc(cfg)
    return _NC_CACHE[cfg]


def kernel(g1, g2, g3, g4, Wh, bh, Wf, bf):
    from concourse import bass_utils

    cfg = FULL
    nc = _get_nc(cfg)
    in_maps = shard_inputs(cfg, g1, g2, g3, g4, Wh, bh, Wf, bf)
    res = bass_utils.run_bass_kernel_spmd(
        nc, in_maps, core_ids=list(range(cfg.n_cores)))
    return unshard_output(cfg, [res.results[c]["out"]
                                for c in range(cfg.n_cores)], bf)
